# revision 7
# baseline (speedup 1.0000x reference)
"""MoE expert-FFN kernel for Trainium2, expert-parallel across 8 NeuronCores.

Problem: out[t] = silu(x[t] @ W1[e_t]^T) @ W2[e_t]^T with
  E=64 experts, D=512, H=1024, T=256 tokens.

v12 strategy (memory-bound on expert weights):
  - Core c owns experts [8c, 8c+8). Host routes tokens to the owning core,
    padding each expert's tokens to capacity C (multiple of 32).
  - Weights quantized to fp8 e3m4 on host: 8.4 MB/core of HBM traffic.
    Power-of-2 scales put weights in e3m4's [0.25, 15.5] normal range;
    the W1 scale is folded into x on the host (h = (x/s1)@(s1*W1)^T is
    exact), the W2 scale is divided out of the output on the host.
    fp8 stationary tiles also halve the PE's LDWEIGHTS time (FWL reads
    4 fp8/cycle), which matters because ldw dominates PE time here.
  - DUAL-RING streaming: all W1 DMAs issued upfront on the sync HWDGE
    ring, all W2 DMAs upfront on the scalar HWDGE ring, so descriptor
    generation for both rings runs ahead of the transfers and the 16 SDMA
    engines round-robin between two loaded queues with no issue gaps.
  - W2s stream in pipeline order so the last-landing transfer feeds the
    last compute stage; outputs are stored as fp16 (host casts back).
  - Weights are the STATIONARY matmul operand; token blocks stream as the
    moving operand. Outputs come out transposed so no PE transpose needed:
       psh[h*128+m, t] = sum_d W1T-tile(c,h)[d,m] * xT[d,t]   (acc over c)
       ht = Silu(psh)                    one fused scalar-engine op, fp16
       psy[d*128+m, t] = sum_h W2T-tile(h,d)[h',m] * ht[h',t] (acc over h)
  - Output stores ride the sync ring (idle after W1s), 8 buffers deep so
    stores never gate compute. Host untransposes + scatters to token order.
"""

import numpy as np

E, D, H, T = 64, 512, 1024, 256
NCORES = 8
EPC = E // NCORES          # experts per core
DC = D // 128              # 4 d-chunks
HC = H // 128              # 8 h-chunks
W1COLS = DC * HC * 128     # 4096 packed stationary cols for fc1
W2COLS = HC * DC * 128     # 4096 packed stationary cols for fc2
WCOLS = W1COLS + W2COLS
CB = 32                    # token capacity granularity

_prog_cache = {}


def _token_blocks(C):
    """Split capacity C into PSUM-bank-sized token blocks (<=64 wide)."""
    blocks = []
    off = 0
    while off < C:
        w = min(64, C - off)
        blocks.append((off, w))
        off += w
    return blocks


def _build_program(C):
    import concourse.mybir as mybir
    import concourse.tile as tile
    from concourse import bacc

    f32 = mybir.dt.float32
    f16 = mybir.dt.float16
    f8 = mybir.dt.float8e3
    nc = bacc.Bacc("TRN2", target_bir_lowering=False, debug=False)

    wts = nc.dram_tensor("wts", [EPC, 128, WCOLS], f8, kind="ExternalInput")
    xt = nc.dram_tensor("xt", [128, EPC * DC * C], f16, kind="ExternalInput")
    yt = nc.dram_tensor("yt", [EPC, 128, DC * C], f16, kind="ExternalOutput")

    blocks = _token_blocks(C)

    with tile.TileContext(nc) as tc:
        with (
            tc.tile_pool(name="w1pool", bufs=EPC) as w1pool,
            tc.tile_pool(name="w2pool", bufs=EPC) as w2pool,
            tc.tile_pool(name="xpool", bufs=1) as xpool,
            tc.tile_pool(name="hpool", bufs=3) as hpool,
            tc.tile_pool(name="ypool", bufs=EPC) as ypool,
            tc.tile_pool(name="psh", bufs=3, space="PSUM") as pshp,
            tc.tile_pool(name="psy", bufs=3, space="PSUM") as psyp,
        ):
            xall = xpool.tile([128, EPC * DC * C], f16)

            w1t = [w1pool.tile([128, W1COLS], f8, tag="w1",
                               name=f"w1_{s}") for s in range(EPC)]
            w2t = [w2pool.tile([128, W2COLS], f8, tag="w2",
                               name=f"w2_{s}") for s in range(EPC)]

            # ---- all weight DMAs upfront, in CONSUMPTION order.  The PE
            # program below runs fc1(0), fc1(1), fc2(0), fc1(2), fc2(1), ...
            # so weights must arrive W1(0), W1(1), W2(0), W1(2), W2(1), ...
            # Alternate items between the two HWDGE rings (sync / scalar):
            # both rings drain concurrently at ~half the 426 GB/s core
            # bandwidth, so global arrival order matches consumption order
            # and both rings finish together.
            #   sync:   x, W1(0), W2(0), W1(2), W2(2), W1(4), W2(4), W1(6), W2(6)
            #   scalar: W1(1), W2(1), W1(3), W2(3), W1(5), W2(5), W1(7), W2(7)
            nc.sync.dma_start(xall[:], xt[:])
            for s in range(0, EPC, 2):
                nc.sync.dma_start(w1t[s][:], wts[s][:, :W1COLS])
                nc.scalar.dma_start(w1t[s + 1][:], wts[s + 1][:, :W1COLS])
                nc.sync.dma_start(w2t[s][:], wts[s][:, W1COLS:])
                nc.scalar.dma_start(w2t[s + 1][:], wts[s + 1][:, W1COLS:])

            def emit_fc1(s):
                """fc1(s) matmuls + silu -> ht(s).  psh[:, h*bw+t] over
                h-tiles; silu is one fused scalar op (fixed cost ~0.26us
                dominates small activations)."""
                w1 = w1t[s]
                hts = []
                for b0, bw in blocks:
                    psh = pshp.tile([128, HC * bw], f32, tag="psh")
                    for h in range(HC):
                        for c in range(DC):
                            nc.tensor.matmul(
                                psh[:, h * bw:(h + 1) * bw],
                                w1[:, (h * DC + c) * 128:(h * DC + c + 1) * 128],
                                xall[:, (s * DC + c) * C + b0:
                                     (s * DC + c) * C + b0 + bw],
                                start=(c == 0),
                                stop=(c == DC - 1),
                            )
                    ht = hpool.tile([128, HC * bw], f16, tag="ht")
                    nc.scalar.activation(
                        ht[:], psh[:], mybir.ActivationFunctionType.Silu
                    )
                    hts.append(ht)
                return hts

            def emit_fc2(s, hts):
                """fc2(s) matmuls + fp16 copy + store (sync ring)."""
                w2 = w2t[s]
                for bi, (b0, bw) in enumerate(blocks):
                    ht = hts[bi]
                    psy = psyp.tile([128, DC * bw], f32, tag="psy")
                    for d in range(DC):
                        for h in range(HC):
                            nc.tensor.matmul(
                                psy[:, d * bw:(d + 1) * bw],
                                w2[:, (d * HC + h) * 128:
                                   (d * HC + h + 1) * 128],
                                ht[:, h * bw:(h + 1) * bw],
                                start=(h == 0),
                                stop=(h == HC - 1),
                            )
                    ybuf = ypool.tile([128, DC * bw], f16, tag="y")
                    nc.vector.tensor_copy(ybuf[:], psy[:])
                    if len(blocks) == 1:
                        nc.sync.dma_start(yt[s], ybuf[:])
                    else:
                        for d in range(DC):
                            nc.sync.dma_start(
                                yt[s][:, d * C + b0: d * C + b0 + bw],
                                ybuf[:, d * bw:(d + 1) * bw],
                            )

            # ---- software-pipelined PE program: fc1(s+1) is emitted before
            # fc2(s), so the silu(s) scalar-engine latency hides under
            # fc1(s+1) instead of stalling the in-order Tensor queue.
            prev_hts = None
            for s in range(EPC):
                hts = emit_fc1(s)
                if prev_hts is not None:
                    emit_fc2(s - 1, prev_hts)
                prev_hts = hts
            emit_fc2(EPC - 1, prev_hts)

    nc.compile()
    return nc


def _route(expert_idx):
    idx = np.asarray(expert_idx).astype(np.int64)
    order = np.argsort(idx, kind="stable")
    counts = np.bincount(idx, minlength=E)
    starts = np.zeros(E + 1, dtype=np.int64)
    starts[1:] = np.cumsum(counts)
    return order, starts, counts


def _pow2_scale(absmax):
    """Largest power of 2 s with absmax*s <= 15.5 (e3m4 finite max)."""
    return 2.0 ** np.floor(np.log2(15.5 / max(absmax, 1e-30)))


def _pack_inputs(x, fc1_w, fc2_w, order, starts, C):
    import ml_dtypes

    f8 = ml_dtypes.float8_e3m4
    s1 = _pow2_scale(np.abs(fc1_w).max())
    s2 = _pow2_scale(np.abs(fc2_w).max())
    global _S2
    _S2 = s2

    in_maps = []
    for core in range(NCORES):
        wh = np.empty((EPC, 128, WCOLS), f8)
        xh = np.zeros((128, EPC * DC * C), np.float16)
        for s in range(EPC):
            e = core * EPC + s
            # fc1 stationary tiles: lhsT(c,h)[k,m] = W1T[c*128+k, h*128+m]
            # packed at col (h*DC + c)*128 + m  -> order [k, h, c, m]
            w1t = fc1_w[e].T * s1  # [D, H]
            wh[s, :, :W1COLS] = (
                w1t.reshape(DC, 128, HC, 128)
                .transpose(1, 2, 0, 3)
                .reshape(128, W1COLS)
                .astype(f8)
            )
            # fc2 stationary tiles: lhsT(h,d)[k,m] = W2T[h*128+k, d*128+m]
            # packed at col (d*HC + h)*128 + m  -> order [k, d, h, m]
            w2t = fc2_w[e].T * s2  # [H, D]
            wh[s, :, W1COLS:] = (
                w2t.reshape(HC, 128, DC, 128)
                .transpose(1, 2, 0, 3)
                .reshape(128, W2COLS)
                .astype(f8)
            )

            toks = order[starts[e]:starts[e + 1]]
            n = len(toks)
            if n:
                # 1/s1 folded into x: h = (x/s1) @ (s1*W1)^T exactly
                xte = np.ascontiguousarray(x[toks].T / s1).reshape(DC, 128, n)
                for c in range(DC):
                    base = (s * DC + c) * C
                    xh[:, base:base + n] = xte[c]
        in_maps.append({"wts": wh, "xt": xh})
    return in_maps


_S2 = 1.0  # W2 scale from the last _pack_inputs; divided out here


def _unpack_outputs(results, order, starts, C, out_dtype):
    out = np.zeros((T, D), out_dtype)
    inv_s2 = np.float32(1.0 / _S2)
    for core in range(NCORES):
        yh = np.asarray(results[core]["yt"]).reshape(EPC, 128, DC, C)
        for s in range(EPC):
            e = core * EPC + s
            toks = order[starts[e]:starts[e + 1]]
            n = len(toks)
            if n:
                # yh[s][p, d, t] = s2 * y[t, d*128+p]
                ye = yh[s].transpose(1, 0, 2).reshape(D, C)
                out[toks] = ye[:, :n].T.astype(out_dtype) * inv_s2
    return out


def kernel(x, expert_idx, fc1_w, fc2_w):
    from concourse.bass_utils import run_bass_kernel_spmd

    x = np.asarray(x, dtype=np.float32)
    fc1_w = np.asarray(fc1_w, dtype=np.float32)
    fc2_w = np.asarray(fc2_w, dtype=np.float32)

    order, starts, counts = _route(expert_idx)
    C = max(CB, int(-(-int(counts.max()) // CB) * CB))

    if C not in _prog_cache:
        _prog_cache[C] = _build_program(C)
    nc = _prog_cache[C]

    in_maps = _pack_inputs(x, fc1_w, fc2_w, order, starts, C)
    res = run_bass_kernel_spmd(nc, in_maps, list(range(NCORES)))
    return _unpack_outputs(res.results, order, starts, C, np.float32)

